# revision 1
# baseline (speedup 1.0000x reference)
"""MiniMax Lightning Attention on 8 Trainium2 NeuronCores.

Sharding: sequence-parallel. Core c handles batch c//4, token chunk
(c%4)*1024..+1024 (4 blocks of 256). The per-block decay-state recurrence
crosses chunk boundaries; each core computes its local per-chunk decay-
weighted KV summary E, an AllGather shares the 8 summaries, and each core
reconstructs its chunk-start state as a decay-weighted sum.

All matmuls run as fp32r (full-rate fp32 on the PE at N>=256).
"""

import numpy as np

from contextlib import ExitStack

import concourse.bacc as bacc
import concourse.mybir as mybir
import concourse.tile as tile
from concourse.bass_utils import run_bass_kernel_spmd
from concourse.masks import make_identity

AF = mybir.ActivationFunctionType
ALU = mybir.AluOpType
F32 = mybir.dt.float32
F32R = mybir.dt.float32r

H = 32
D = 64
BS = 256
HID = 2048
B = 2
S = 4096
NC = 8
T = S // 4            # tokens per core (1024)
NCH = T // 128        # 8 token chunks of 128
NBLK = T // BS        # 4 blocks per core
LAYER_IDX = 0
NUM_LAYERS = 32
EPS = 1e-5


def _decay():
    base = 1.0 / 2.0 ** (8.0 / H)
    rate = base ** (np.arange(H, dtype=np.float64) + 1.0)
    factor = 1.0 - LAYER_IDX / (NUM_LAYERS - 1 + 1e-5) + 1e-5
    slope = rate * factor                                  # (H,)
    r = np.arange(BS, dtype=np.float64) + 1.0
    qd = np.exp(-slope[:, None] * r[None, :])              # (H, BS) query decay
    kd = np.exp(-slope[:, None] * (BS - r[None, :]))       # (H, BS) key decay
    ij = r[:, None] - r[None, :]                           # i - j
    dd = np.where(
        ij[None] >= 0, np.exp(-slope[:, None, None] * ij[None]), 0.0
    )                                                      # (H, BS_i, BS_j)
    bd = np.exp(-slope * BS)                               # (H,) block decay
    return slope, qd, kd, dd, bd


def _build_nc():
    nc = bacc.Bacc(num_devices=NC)
    hsT = nc.declare_dram_parameter("hsT", [HID, T], F32R, isOutput=False)
    wqkT = nc.declare_dram_parameter("wqkT", [HID, 2 * H * D], F32R, isOutput=False)
    wvT = nc.declare_dram_parameter("wvT", [HID, H * D], F32R, isOutput=False)
    gwT = nc.declare_dram_parameter("gwT", [HID, HID], F32R, isOutput=False)
    owT = nc.declare_dram_parameter("owT", [H * D, HID], F32R, isOutput=False)
    ddm = nc.declare_dram_parameter("ddm", [H, 2, 128, BS], F32, isOutput=False)
    qdm = nc.declare_dram_parameter("qdm", [H, D, BS], F32, isOutput=False)
    kdm = nc.declare_dram_parameter("kdm", [128, 2 * H], F32, isOutput=False)
    nw = nc.declare_dram_parameter("nw", [128, 16], F32, isOutput=False)
    swm = nc.declare_dram_parameter("swm", [D, H * NC], F32, isOutput=False)
    out = nc.declare_dram_parameter("out", [T, HID], F32, isOutput=True)

    qk_spill = nc.dram_tensor("qk_spill", [H, 2, D, T], F32R)
    gate_spill = nc.dram_tensor("gate_spill", [16, 128, T], F32)
    attn_spill = nc.dram_tensor("attn_spill", [16, 128, T], F32R)
    c_dram = nc.dram_tensor("c_dram", [H, D, NBLK * D], F32)
    v_dram = nc.dram_tensor("v_dram", [NCH, 128, H * D], F32R)
    eloc = nc.dram_tensor("eloc", [H, D, D], F32)
    egath = nc.dram_tensor("egath", [NC, H, D, D], F32, addr_space="Shared")
    ssq_rt = nc.dram_tensor("ssq_rt", [T], F32)

    bd_f = [float(x) for x in _decay()[4]]

    with tile.TileContext(nc, pool_alloc_mode="stack") as tc:
        # ---- constants + resident tensors -------------------------------
        ident, free_ident = tc.tile([128, 128], F32, name="ident")
        make_identity(nc, ident[:])
        ones_f, free_ones_f = tc.tile([128, 1], F32, name="ones_f")
        nc.vector.memset(ones_f[:], 1.0)
        ones, free_ones = tc.tile([128, 1], F32R, name="ones")
        nc.scalar.copy(ones[:], ones_f[:])
        eps_sb, free_eps = tc.tile([128, 1], F32, name="eps_sb")
        nc.vector.memset(eps_sb[:], EPS)
        nw_sb, free_nw = tc.tile([128, 16], F32, name="nw_sb")
        nc.sync.dma_start(nw_sb[:], nw[:])
        kdm_sb, free_kdm = tc.tile([128, 2 * H], F32, name="kdm_sb")
        nc.sync.dma_start(kdm_sb[:], kdm[:])
        swm_sb, free_swm = tc.tile([D, H * NC], F32, name="swm_sb")
        nc.sync.dma_start(swm_sb[:], swm[:])

        _xt_ctx = ExitStack()
        xt_pool = _xt_ctx.enter_context(tc.tile_pool(name="xt_pool", bufs=1))
        xT = xt_pool.tile([128, 16, T], F32R, name="xT")
        for k in range(16):
            nc.sync.dma_start(xT[:, k, :], hsT[k * 128 : (k + 1) * 128, :])
        _v_ctx = ExitStack()
        v_pool = _v_ctx.enter_context(tc.tile_pool(name="v_pool", bufs=1))
        V_sb = v_pool.tile([128, NCH, H * D], F32R, name="V_sb")

        # ---- phase V: value projection (tok-major, all heads) -----------
        with tc.tile_pool(name="wv_p", bufs=3) as wv_p, tc.tile_pool(
            name="ps_v", bufs=1, space="PSUM"
        ) as ps_v:
            for n in range(4):
                pv = [
                    ps_v.tile([128, 512], F32, name=f"pv{m}") for m in range(NCH)
                ]
                for k in range(16):
                    wv_t = wv_p.tile([128, 512], F32R, name="wv_t")
                    nc.sync.dma_start(
                        wv_t[:], wvT[k * 128 : (k + 1) * 128, n * 512 : (n + 1) * 512]
                    )
                    for m in range(NCH):
                        nc.tensor.matmul(
                            pv[m][:],
                            xT[:, k, m * 128 : (m + 1) * 128],
                            wv_t[:],
                            start=(k == 0),
                            stop=(k == 15),
                        )
                for m in range(NCH):
                    nc.scalar.activation(
                        V_sb[:, m, n * 512 : (n + 1) * 512], pv[m][:], AF.Silu
                    )

        # ---- phase QK: q/k projection (dim-major per head) + contribs ---
        with tc.tile_pool(name="wqk_p", bufs=2) as wqk_p, tc.tile_pool(
            name="qk_p", bufs=2
        ) as qk_p, tc.tile_pool(name="tok_p", bufs=2) as tok_p, tc.tile_pool(
            name="ce_p", bufs=2
        ) as ce_p, tc.tile_pool(
            name="ps_qk", bufs=2, space="PSUM"
        ) as ps_qk, tc.tile_pool(
            name="ps_t", bufs=2, space="PSUM"
        ) as ps_t, tc.tile_pool(
            name="ps_c", bufs=2, space="PSUM"
        ) as ps_c:
            for h in range(H):
                wqk_t = wqk_p.tile([128, 16, 128], F32R, name="wqk_t")
                nc.sync.dma_start(
                    wqk_t[:],
                    wqkT[:, h * 128 : (h + 1) * 128].rearrange(
                        "(ko p) m -> p ko m", p=128
                    ),
                )
                pqk = ps_qk.tile([128, 2, 512], F32, name="pqk")
                for n in range(2):
                    for k in range(16):
                        nc.tensor.matmul(
                            pqk[:, n, :],
                            wqk_t[:, k, :],
                            xT[:, k, n * 512 : (n + 1) * 512],
                            start=(k == 0),
                            stop=(k == 15),
                        )
                qT_t = qk_p.tile([D, T], F32R, name="qT_t")
                kT_t = qk_p.tile([D, T], F32R, name="kT_t")
                nc.scalar.activation(
                    qT_t[:], pqk[0:D].rearrange("p n f -> p (n f)"), AF.Silu
                )
                nc.scalar.activation(
                    kT_t[:], pqk[D:128].rearrange("p n f -> p (n f)"), AF.Silu
                )
                nc.sync.dma_start(qk_spill[h, 0], qT_t[:])
                nc.sync.dma_start(qk_spill[h, 1], kT_t[:])

                # k back to tok-major via PE transpose
                k_tok = tok_p.tile([128, NCH, D], F32R, name="k_tok")
                for m in range(NCH):
                    pst = ps_t.tile([128, D], F32, name="pst")
                    nc.tensor.transpose(
                        pst[:],
                        kT_t[:, m * 128 : (m + 1) * 128].bitcast(F32),
                        ident[0:D, 0:D],
                    )
                    nc.scalar.copy(k_tok[:, m, :], pst[:])
                # v scaled by key-decay
                v_kd = tok_p.tile([128, NCH, D], F32R, name="v_kd")
                for m in range(NCH):
                    nc.vector.tensor_scalar_mul(
                        v_kd[:, m, :],
                        V_sb[:, m, h * D : (h + 1) * D],
                        kdm_sb[:, 2 * h + (m % 2) : 2 * h + (m % 2) + 1],
                    )
                # block contributions C_jb = (k*kd)^T v and chunk summary E
                c_st = ce_p.tile([D, NBLK, D], F32, name="c_st")
                for jb in range(NBLK):
                    pc = ps_c.tile([D, D], F32, name="pc")
                    for half in range(2):
                        m = 2 * jb + half
                        nc.tensor.matmul(
                            pc[:],
                            k_tok[:, m, :],
                            v_kd[:, m, :],
                            start=(half == 0),
                            stop=(half == 1),
                        )
                    nc.scalar.copy(c_st[:, jb, :], pc[:])
                nc.sync.dma_start(c_dram[h], c_st[:].rearrange("d b e -> d (b e)"))
                e_t = ce_p.tile([D, D], F32, name="e_t")
                nc.vector.tensor_copy(e_t[:], c_st[:, 0, :])
                for jb in range(1, NBLK):
                    nc.vector.scalar_tensor_tensor(
                        e_t[:], e_t[:], bd_f[h], c_st[:, jb, :], ALU.mult, ALU.add
                    )
                nc.sync.dma_start(eloc[h], e_t[:])

        for m in range(NCH):
            nc.sync.dma_start(v_dram[m], V_sb[:, m, :])
        _v_ctx.close()

        # ---- collective: share per-chunk KV summaries -------------------
        nc.gpsimd.collective_compute(
            "AllGather",
            ALU.bypass,
            replica_groups=[list(range(NC))],
            ins=[eloc[:]],
            outs=[egath[:]],
        )

        # ---- phase A: attention per head --------------------------------
        with tc.tile_pool(name="aq_p", bufs=2) as aq_p, tc.tile_pool(
            name="am_p", bufs=2
        ) as am_p, tc.tile_pool(name="ss_p", bufs=3) as ss_p, tc.tile_pool(
            name="ys_p", bufs=3
        ) as ys_p, tc.tile_pool(name="vh_p", bufs=2) as vh_p, tc.tile_pool(
            name="gw_p", bufs=2
        ) as gw_p, tc.tile_pool(name="go_p", bufs=2) as go_p, tc.tile_pool(
            name="ps_aw", bufs=2, space="PSUM"
        ) as ps_aw, tc.tile_pool(
            name="ps_ys", bufs=2, space="PSUM"
        ) as ps_ys, tc.tile_pool(
            name="ps_g", bufs=2, space="PSUM"
        ) as ps_g:
            for h in range(H):
                qT_a = aq_p.tile([D, T], F32R, name="qT_a")
                kT_a = aq_p.tile([D, T], F32R, name="kT_a")
                nc.sync.dma_start(qT_a[:], qk_spill[h, 0])
                nc.sync.dma_start(kT_a[:], qk_spill[h, 1])
                dd_t = am_p.tile([128, 2, BS], F32, name="dd_t")
                nc.sync.dma_start(dd_t[:], ddm[h].rearrange("c p i -> p c i"))
                qd_t = am_p.tile([D, BS], F32, name="qd_t")
                nc.sync.dma_start(qd_t[:], qdm[h])
                eg_t = am_p.tile([D, NC, D], F32, name="eg_t")
                nc.sync.dma_start(eg_t[:], egath[:, h, :, :].rearrange("c d e -> d c e"))
                c_a = am_p.tile([D, NBLK, D], F32, name="c_a")
                nc.sync.dma_start(c_a[:], c_dram[h].rearrange("d (b e) -> d b e", b=NBLK))
                v_h = vh_p.tile([128, NCH, D], F32R, name="v_h")
                nc.sync.dma_start(
                    v_h[:],
                    v_dram[:, :, h * D : (h + 1) * D].rearrange("m p e -> p m e"),
                )

                qdq = aq_p.tile([D, NBLK, BS], F32R, name="qdq")
                for jb in range(NBLK):
                    nc.vector.tensor_mul(
                        qdq[:, jb, :],
                        qT_a[:, jb * BS : (jb + 1) * BS].bitcast(F32),
                        qd_t[:],
                    )
                ss = ss_p.tile([D, D], F32R, name="ss")
                nc.vector.tensor_scalar_mul(
                    ss[:], eg_t[:, 0, :], swm_sb[:, h * NC : h * NC + 1]
                )
                for cc in range(1, NC):
                    nc.vector.scalar_tensor_tensor(
                        ss[:],
                        eg_t[:, cc, :],
                        swm_sb[:, h * NC + cc : h * NC + cc + 1],
                        ss[:],
                        ALU.mult,
                        ALU.add,
                    )
                ys_st = ys_p.tile([D, NBLK, BS], F32R, name="ys_st")
                for jb in range(NBLK):
                    paw = ps_aw.tile([128, 2, BS], F32, name="paw")
                    for jc in range(2):
                        nc.tensor.matmul(
                            paw[:, jc, :],
                            kT_a[:, jb * BS + jc * 128 : jb * BS + (jc + 1) * 128],
                            qT_a[:, jb * BS : (jb + 1) * BS],
                            start=True,
                            stop=True,
                        )
                    awm = ys_p.tile([128, 2, BS], F32R, name="awm")
                    nc.vector.tensor_mul(awm[:], paw[:], dd_t[:])
                    pys = ps_ys.tile([D, BS], F32, name="pys")
                    nc.tensor.matmul(
                        pys[:], ss[:], qdq[:, jb, :], start=True, stop=False
                    )
                    for jc in range(2):
                        nc.tensor.matmul(
                            pys[:],
                            v_h[:, 2 * jb + jc, :],
                            awm[:, jc, :],
                            start=False,
                            stop=(jc == 1),
                        )
                    nc.scalar.copy(ys_st[:, jb, :], pys[:])
                    if jb < NBLK - 1:
                        ss2 = ss_p.tile([D, D], F32R, name="ss")
                        nc.vector.scalar_tensor_tensor(
                            ss2[:], ss[:], bd_f[h], c_a[:, jb, :], ALU.mult, ALU.add
                        )
                        ss = ss2
                nc.sync.dma_start(
                    attn_spill[h // 2, (h % 2) * D : (h % 2 + 1) * D, :],
                    ys_st[:].rearrange("d b i -> d (b i)"),
                )
                # interleaved gate-projection chunk: keeps PE warm+dense
                if h % 2 == 1:
                    gm = h // 2
                    gw_t = gw_p.tile([128, 16, 128], F32R, name="gw_t")
                    nc.sync.dma_start(
                        gw_t[:],
                        gwT[:, gm * 128 : (gm + 1) * 128].rearrange(
                            "(ko p) g -> p ko g", p=128
                        ),
                    )
                    for gn in range(2):
                        pg = ps_g.tile([128, 512], F32, name="pg")
                        for gk in range(16):
                            nc.tensor.matmul(
                                pg[:],
                                gw_t[:, gk, :],
                                xT[:, gk, gn * 512 : (gn + 1) * 512],
                                start=(gk == 0),
                                stop=(gk == 15),
                            )
                        go_t = go_p.tile([128, 512], F32, name="go_t")
                        nc.scalar.activation(go_t[:], pg[:], AF.Sigmoid)
                        nc.sync.dma_start(
                            gate_spill[gm, :, gn * 512 : (gn + 1) * 512], go_t[:]
                        )
        _xt_ctx.close()

        # ---- phase F: rmsnorm + gate + output projection ----------------
        _g_ctx = ExitStack()
        g_pool = _g_ctx.enter_context(tc.tile_pool(name="g_pool", bufs=1))
        gate_sb = g_pool.tile([128, 16, T], F32, name="gate_sb")
        for c in range(16):
            nc.sync.dma_start(gate_sb[:, c, :], gate_spill[c])
        with tc.tile_pool(name="sq_p", bufs=2) as sq_p, tc.tile_pool(
            name="an_p", bufs=3
        ) as an_p:
          with tc.tile_pool(name="ps_sq", bufs=1, space="PSUM") as ps_sq:
            ssq0 = ps_sq.tile([1, 512], F32, name="ssq0")
            ssq1 = ps_sq.tile([1, 512], F32, name="ssq1")
            for c in range(16):
                at = an_p.tile([128, T], F32R, name="at")
                nc.sync.dma_start(at[:], attn_spill[c])
                sq = sq_p.tile([128, T], F32R, name="sq")
                nc.scalar.activation(sq[:], at[:].bitcast(F32), AF.Square)
                for half in range(2):
                    nc.tensor.matmul(
                        [ssq0, ssq1][half][:],
                        ones[:],
                        sq[:, half * 512 : (half + 1) * 512],
                        start=(c == 0),
                        stop=(c == 15),
                        skip_group_check=True,
                    )
                nc.vector.scalar_tensor_tensor(
                    gate_sb[:, c, :].bitcast(F32R),
                    at[:].bitcast(F32),
                    nw_sb[:, c : c + 1],
                    gate_sb[:, c, :],
                    ALU.mult,
                    ALU.mult,
                )
            ssq_sb = sq_p.tile([1, T], F32, name="ssq_sb")
            nc.vector.tensor_copy(ssq_sb[:, 0:512], ssq0[:])
            nc.vector.tensor_copy(ssq_sb[:, 512:1024], ssq1[:])
            nc.sync.dma_start(ssq_rt[:], ssq_sb[:])
          if True:
            ns_l = sq_p.tile([128, NCH], F32, name="ns_l")
            nc.sync.dma_start(ns_l[:], ssq_rt.rearrange("(c p) -> p c", p=128))
            ns_t = sq_p.tile([128, NCH], F32, name="ns_t")
            nc.scalar.activation(
                ns_t[:], ns_l[:], AF.Sqrt, bias=eps_sb[:, 0:1], scale=1.0 / (H * D)
            )
            ns_sb = sq_p.tile([128, NCH], F32, name="ns_sb")
            nc.vector.reciprocal(ns_sb[:], ns_t[:])

            with tc.tile_pool(name="ow_p", bufs=3) as ow_p, tc.tile_pool(
                name="oo_p", bufs=3
            ) as oo_p, tc.tile_pool(name="ps_o", bufs=1, space="PSUM") as ps_o:
                for n in range(4):
                    po = [
                        ps_o.tile([128, 512], F32, name=f"po{m}") for m in range(NCH)
                    ]
                    for k in range(16):
                        ow_t = ow_p.tile([128, 512], F32R, name="ow_t")
                        nc.sync.dma_start(
                            ow_t[:],
                            owT[k * 128 : (k + 1) * 128, n * 512 : (n + 1) * 512],
                        )
                        for m in range(NCH):
                            nc.tensor.matmul(
                                po[m][:],
                                gate_sb[:, k, m * 128 : (m + 1) * 128].bitcast(F32R),
                                ow_t[:],
                                start=(k == 0),
                                stop=(k == 15),
                            )
                    for m in range(NCH):
                        oo_t = oo_p.tile([128, 512], F32, name="oo_t")
                        nc.scalar.mul(oo_t[:], po[m][:], ns_sb[:, m : m + 1])
                        nc.sync.dma_start(
                            out[m * 128 : (m + 1) * 128, n * 512 : (n + 1) * 512],
                            oo_t[:],
                        )
        _g_ctx.close()
        free_swm()
        free_kdm()
        free_nw()
        free_eps()
        free_ones()
        free_ones_f()
        free_ident()
    nc.finalize()
    return nc


_CACHE = {}


def _get_nc():
    if "nc" not in _CACHE:
        _CACHE["nc"] = _build_nc()
    return _CACHE["nc"]


def _host_prep(hidden_states, qkv_w, out_w, gate_w, norm_w):
    slope, qd, kd, dd, bd = _decay()
    w3 = qkv_w.reshape(H, 3 * D, HID)
    wq = w3[:, 0:D, :]
    wk = w3[:, D : 2 * D, :]
    wv = w3[:, 2 * D : 3 * D, :]
    wqk = np.concatenate([wq, wk], axis=1).reshape(2 * H * D, HID)
    wqkT = np.ascontiguousarray(wqk.T, dtype=np.float32)
    wvT = np.ascontiguousarray(wv.reshape(H * D, HID).T, dtype=np.float32)
    gwT = np.ascontiguousarray(gate_w.T, dtype=np.float32)
    owT = np.ascontiguousarray(out_w.T, dtype=np.float32)
    ddm = np.ascontiguousarray(
        dd.transpose(0, 2, 1).reshape(H, 2, 128, BS), dtype=np.float32
    )
    qdm = np.ascontiguousarray(
        np.broadcast_to(qd[:, None, :], (H, D, BS)), dtype=np.float32
    )
    kdm = np.ascontiguousarray(
        kd.reshape(H, 2, 128).transpose(2, 0, 1).reshape(128, 2 * H), dtype=np.float32
    )
    nw = np.ascontiguousarray(norm_w.reshape(16, 128).T, dtype=np.float32)

    shared = dict(wqkT=wqkT, wvT=wvT, gwT=gwT, owT=owT, ddm=ddm, qdm=qdm,
                  kdm=kdm, nw=nw)
    in_maps = []
    for c in range(NC):
        bb, p = c // 4, c % 4
        hsT = np.ascontiguousarray(
            hidden_states[bb, p * T : (p + 1) * T, :].T, dtype=np.float32
        )
        sw = np.zeros((H, NC), dtype=np.float64)
        for cc in range(NC):
            if cc // 4 == bb and cc % 4 < p:
                sw[:, cc] = bd ** (4.0 * (p - 1 - (cc % 4)))
        swm = np.ascontiguousarray(
            np.broadcast_to(sw.reshape(1, H * NC), (D, H * NC)), dtype=np.float32
        )
        in_maps.append(dict(hsT=hsT, swm=swm, **shared))
    return in_maps


def _run(inputs, trace=False):
    nc = _get_nc()
    in_maps = _host_prep(
        np.asarray(inputs["hidden_states"], dtype=np.float32),
        np.asarray(inputs["qkv_w"], dtype=np.float32),
        np.asarray(inputs["out_w"], dtype=np.float32),
        np.asarray(inputs["gate_w"], dtype=np.float32),
        np.asarray(inputs["norm_w"], dtype=np.float32),
    )
    res = run_bass_kernel_spmd(nc, in_maps, core_ids=list(range(NC)), trace=trace)
    full = np.empty((B, S, HID), dtype=np.float32)
    for c in range(NC):
        bb, p = c // 4, c % 4
        full[bb, p * T : (p + 1) * T, :] = res.results[c]["out"]
    return full, res


def kernel(**inputs):
    return _run(inputs, trace=False)[0]


def kernel_traced(**inputs):
    full, res = _run(inputs, trace=True)
    return full, res.exec_time_ns



# revision 5
# speedup vs baseline: 1.5232x; 1.5232x over previous
"""MiniMax Lightning Attention on 8 Trainium2 NeuronCores — bf16 edition.

Sharding: sequence-parallel. Core c handles batch c//4, token chunk
(c%4)*1024..+1024 (4 blocks of 256). Per-chunk decay-weighted KV summaries
are AllGathered within each batch's 4-core group; each core reconstructs
its chunk-start state as a decay-weighted sum.

All matmuls run in bf16 (fp32 PSUM accumulation). Heads are processed in
pairs packed onto partition halves (head 2c on partitions 0-63, head 2c+1
on 64-127); the K=64 / M=64 attention matmuls for the two heads execute
concurrently on disjoint PE row/col groups via tile_position auto-derive.
q/k/attn stay SBUF-resident; only the gate activations round-trip DRAM.
"""

import numpy as np
import ml_dtypes

from contextlib import ExitStack

import concourse.bacc as bacc
import concourse.mybir as mybir
import concourse.tile as tile
from concourse.bass_utils import run_bass_kernel_spmd
from concourse.masks import make_identity

AF = mybir.ActivationFunctionType
ALU = mybir.AluOpType
F32 = mybir.dt.float32
BF = mybir.dt.bfloat16
BF_NP = ml_dtypes.bfloat16

H = 32
D = 64
BS = 256
HID = 2048
B = 2
S = 4096
NC = 8
T = S // 4            # tokens per core (1024)
NCH = T // 128        # 8 token chunks of 128
NBLK = T // BS        # 4 blocks of 256 per core
NP = H // 2           # 16 head pairs
LAYER_IDX = 0
NUM_LAYERS = 32
EPS = 1e-5


def _decay():
    base = 1.0 / 2.0 ** (8.0 / H)
    rate = base ** (np.arange(H, dtype=np.float64) + 1.0)
    factor = 1.0 - LAYER_IDX / (NUM_LAYERS - 1 + 1e-5) + 1e-5
    slope = rate * factor                                  # (H,)
    r = np.arange(BS, dtype=np.float64) + 1.0
    qd = np.exp(-slope[:, None] * r[None, :])              # (H, BS) query decay
    kd = np.exp(-slope[:, None] * (BS - r[None, :]))       # (H, BS) key decay
    ij = r[:, None] - r[None, :]                           # i - j
    dd = np.where(
        ij[None] >= 0, np.exp(-slope[:, None, None] * ij[None]), 0.0
    )                                                      # (H, BS_i, BS_j)
    bd = np.exp(-slope * BS)                               # (H,) block decay
    return slope, qd, kd, dd, bd


def _build_nc():
    nc = bacc.Bacc(num_devices=NC)
    hsT = nc.declare_dram_parameter("hsT", [HID, T], BF, isOutput=False)
    wqT = nc.declare_dram_parameter("wqT", [HID, H * D], BF, isOutput=False)
    wkT = nc.declare_dram_parameter("wkT", [HID, H * D], BF, isOutput=False)
    wvT = nc.declare_dram_parameter("wvT", [HID, H * D], BF, isOutput=False)
    gwT = nc.declare_dram_parameter("gwT", [HID, HID], BF, isOutput=False)
    owT = nc.declare_dram_parameter("owT", [H * D, HID], BF, isOutput=False)
    ddm = nc.declare_dram_parameter("ddm", [NP, 128, 2, 2, BS], F32, isOutput=False)
    qdm = nc.declare_dram_parameter("qdm", [128, NP, BS], BF, isOutput=False)
    kdm = nc.declare_dram_parameter("kdm", [128, 2 * H], F32, isOutput=False)
    nw = nc.declare_dram_parameter("nw", [128, 16], F32, isOutput=False)
    swm = nc.declare_dram_parameter("swm", [128, NP * 8], F32, isOutput=False)
    bdm = nc.declare_dram_parameter("bdm", [128, NP, NBLK], F32, isOutput=False)
    out = nc.declare_dram_parameter("out", [T, HID], F32, isOutput=True)

    gate_spill = nc.dram_tensor("gate_spill", [NP, 128, T], BF)
    eloc = nc.dram_tensor("eloc", [NP, 128, D], F32)
    egath = nc.dram_tensor("egath", [NC, NP, 128, D], F32, addr_space="Shared")
    ssq_rt = nc.dram_tensor("ssq_rt", [T], F32)

    with tile.TileContext(nc, pool_alloc_mode="stack") as tc:
        # ---- constants ---------------------------------------------------
        ident_b, free_ident = tc.tile([128, 128], BF, name="ident_b")
        make_identity(nc, ident_b[:])
        ones_f, free_ones_f = tc.tile([128, 1], F32, name="ones_f")
        nc.vector.memset(ones_f[:], 1.0)
        ones_b, free_ones_b = tc.tile([128, 1], BF, name="ones_b")
        nc.scalar.copy(ones_b[:], ones_f[:])
        eps_sb, free_eps = tc.tile([128, 1], F32, name="eps_sb")
        nc.vector.memset(eps_sb[:], EPS)
        nw_sb, free_nw = tc.tile([128, 16], F32, name="nw_sb")
        nc.sync.dma_start(nw_sb[:], nw[:])
        kdm_sb, free_kdm = tc.tile([128, 2 * H], F32, name="kdm_sb")
        nc.sync.dma_start(kdm_sb[:], kdm[:])
        swm_sb, free_swm = tc.tile([128, NP * 8], F32, name="swm_sb")
        nc.sync.dma_start(swm_sb[:], swm[:])
        bdm_sb, free_bdm = tc.tile([128, NP, NBLK], F32, name="bdm_sb")
        nc.sync.dma_start(bdm_sb[:], bdm[:])

        # ---- long-lived residents (stack order matters: LIFO close) ------
        _v_ctx = ExitStack()
        v_pool = _v_ctx.enter_context(tc.tile_pool(name="v_pool", bufs=1))
        V_sb = v_pool.tile([128, NCH, H * D], BF, name="V_sb")

        _pf_ctx = ExitStack()
        pf_pool = _pf_ctx.enter_context(tc.tile_pool(name="pf_pool", bufs=1))
        prefix_sb = pf_pool.tile([128, NP, NBLK - 1, D], BF, name="prefix_sb")

        _ys_ctx = ExitStack()
        ys_pool = _ys_ctx.enter_context(tc.tile_pool(name="ys_pool", bufs=1))
        ys_sb = ys_pool.tile([128, 16, T], BF, name="ys_sb")

        _qk_ctx = ExitStack()
        qk_pool = _qk_ctx.enter_context(tc.tile_pool(name="qk_pool", bufs=1))
        qT_sb = qk_pool.tile([128, NP, T], BF, name="qT_sb")
        kT_sb = qk_pool.tile([128, NP, T], BF, name="kT_sb")

        _qd_ctx = ExitStack()
        qd_pool = _qd_ctx.enter_context(tc.tile_pool(name="qd_pool", bufs=1))
        qdm_sb = qd_pool.tile([128, NP, BS], BF, name="qdm_sb")
        nc.sync.dma_start(qdm_sb[:], qdm[:])

        _xt_ctx = ExitStack()
        xt_pool = _xt_ctx.enter_context(tc.tile_pool(name="xt_pool", bufs=1))
        xT = xt_pool.tile([128, 16, T], BF, name="xT")
        for g in range(4):
            nc.sync.dma_start(
                xT[:, 4 * g : 4 * (g + 1), :],
                hsT[g * 512 : (g + 1) * 512, :].rearrange(
                    "(ko p) t -> p ko t", p=128
                ),
            )

        # ---- phase V: value projection (tok-major, all heads) ------------
        with tc.tile_pool(name="wv_p", bufs=3) as wv_p, tc.tile_pool(
            name="ps_v", bufs=1, space="PSUM"
        ) as ps_v:
            for n in range(4):
                pv = [
                    ps_v.tile([128, 512], F32, name=f"pv{m}") for m in range(NCH)
                ]
                for k in range(16):
                    wv_t = wv_p.tile([128, 512], BF, name="wv_t")
                    nc.sync.dma_start(
                        wv_t[:], wvT[k * 128 : (k + 1) * 128, n * 512 : (n + 1) * 512]
                    )
                    for m in range(NCH):
                        nc.tensor.matmul(
                            pv[m][:],
                            xT[:, k, m * 128 : (m + 1) * 128],
                            wv_t[:],
                            start=(k == 0),
                            stop=(k == 15),
                        )
                for m in range(NCH):
                    nc.scalar.activation(
                        V_sb[:, m, n * 512 : (n + 1) * 512], pv[m][:], AF.Silu
                    )

        # ---- phase QK: q/k projection per head pair + chunk summaries ----
        with tc.tile_pool(name="wq_p", bufs=2) as wq_p, tc.tile_pool(
            name="wk_p", bufs=2
        ) as wk_p, tc.tile_pool(name="kt_p", bufs=2) as kt_p, tc.tile_pool(
            name="vk_p", bufs=2
        ) as vk_p, tc.tile_pool(
            name="ef_p", bufs=2
        ) as ef_p, tc.tile_pool(
            name="ps_qk", bufs=2, space="PSUM"
        ) as ps_qk, tc.tile_pool(
            name="ps_t", bufs=1, space="PSUM"
        ) as ps_t, tc.tile_pool(
            name="ps_c", bufs=2, space="PSUM"
        ) as ps_c:
            for c in range(NP):
                hA, hB = 2 * c, 2 * c + 1
                wq_t = wq_p.tile([128, 16, 128], BF, name="wq_t")
                nc.sync.dma_start(
                    wq_t[:],
                    wqT[:, c * 128 : (c + 1) * 128].rearrange(
                        "(ko p) m -> p ko m", p=128
                    ),
                )
                wk_t = wk_p.tile([128, 16, 128], BF, name="wk_t")
                nc.sync.dma_start(
                    wk_t[:],
                    wkT[:, c * 128 : (c + 1) * 128].rearrange(
                        "(ko p) m -> p ko m", p=128
                    ),
                )
                for n in range(2):
                    pq = ps_qk.tile([128, 512], F32, name="pq")
                    for k in range(16):
                        nc.tensor.matmul(
                            pq[:],
                            wq_t[:, k, :],
                            xT[:, k, n * 512 : (n + 1) * 512],
                            start=(k == 0),
                            stop=(k == 15),
                        )
                    nc.scalar.activation(
                        qT_sb[:, c, n * 512 : (n + 1) * 512], pq[:], AF.Silu
                    )
                    pk = ps_qk.tile([128, 512], F32, name="pk")
                    for k in range(16):
                        nc.tensor.matmul(
                            pk[:],
                            wk_t[:, k, :],
                            xT[:, k, n * 512 : (n + 1) * 512],
                            start=(k == 0),
                            stop=(k == 15),
                        )
                    nc.scalar.activation(
                        kT_sb[:, c, n * 512 : (n + 1) * 512], pk[:], AF.Silu
                    )
                # k back to tok-major via PE transpose (paired row groups)
                pstA = ps_t.tile([128, 512], BF, name="pstA",
                                 padded_shape=[128, 1024])
                pstB = ps_t.tile([128, 512], BF, name="pstB",
                                 padded_shape=[128, 1024])
                for m in range(NCH):
                    nc.tensor.transpose(
                        pstA[:, m * 64 : (m + 1) * 64],
                        kT_sb[0:64, c, m * 128 : (m + 1) * 128],
                        ident_b[0:64, 0:64],
                    )
                    nc.tensor.transpose(
                        pstB[:, m * 64 : (m + 1) * 64],
                        kT_sb[64:128, c, m * 128 : (m + 1) * 128],
                        ident_b[64:128, 64:128],
                    )
                k_tokA = kt_p.tile([128, NCH, D], BF, name="k_tokA")
                k_tokB = kt_p.tile([128, NCH, D], BF, name="k_tokB")
                nc.scalar.copy(k_tokA[:].rearrange("p m d -> p (m d)"), pstA[:])
                nc.scalar.copy(k_tokB[:].rearrange("p m d -> p (m d)"), pstB[:])
                # v scaled by key-decay
                v_kdA = vk_p.tile([128, NCH, D], BF, name="v_kdA")
                v_kdB = vk_p.tile([128, NCH, D], BF, name="v_kdB")
                for m in range(NCH):
                    nc.vector.tensor_scalar_mul(
                        v_kdA[:, m, :],
                        V_sb[:, m, hA * D : (hA + 1) * D],
                        kdm_sb[:, 2 * hA + (m % 2) : 2 * hA + (m % 2) + 1],
                    )
                    nc.vector.tensor_scalar_mul(
                        v_kdB[:, m, :],
                        V_sb[:, m, hB * D : (hB + 1) * D],
                        kdm_sb[:, 2 * hB + (m % 2) : 2 * hB + (m % 2) + 1],
                    )
                # block contributions C_jb (paired col groups)
                pc = ps_c.tile([128, NBLK, D], F32, name="pc",
                               padded_shape=[128, NBLK, 128])
                for jb in range(NBLK):
                    for half in range(2):
                        m = 2 * jb + half
                        nc.tensor.matmul(
                            pc[0:64, jb, :],
                            k_tokA[:, m, :],
                            v_kdA[:, m, :],
                            start=(half == 0),
                            stop=(half == 1),
                        )
                        nc.tensor.matmul(
                            pc[64:128, jb, :],
                            k_tokB[:, m, :],
                            v_kdB[:, m, :],
                            start=(half == 0),
                            stop=(half == 1),
                        )
                # decay-prefix chain (both heads at once; f32)
                e_f = ef_p.tile([128, D], F32, name="e_f")
                nc.vector.tensor_copy(e_f[:], pc[:, 0, :])
                nc.scalar.copy(prefix_sb[:, c, 0, :], e_f[:])
                for jb in range(1, NBLK):
                    nc.vector.scalar_tensor_tensor(
                        e_f[:], e_f[:], bdm_sb[:, c, 1:2], pc[:, jb, :],
                        ALU.mult, ALU.add,
                    )
                    if jb < NBLK - 1:
                        nc.scalar.copy(prefix_sb[:, c, jb, :], e_f[:])
                nc.sync.dma_start(eloc[c], e_f[:])

        # ---- collective: share per-chunk KV summaries (4-core groups) ----
        nc.gpsimd.collective_compute(
            "AllGather",
            ALU.bypass,
            replica_groups=[list(range(NC))],
            ins=[eloc[:]],
            outs=[egath[:]],
        )

        # ---- gate projection (overlaps the collective) -------------------
        with tc.tile_pool(name="gw_p", bufs=2) as gw_p, tc.tile_pool(
            name="gg_p", bufs=3
        ) as gg_p, tc.tile_pool(name="ps_g", bufs=2, space="PSUM") as ps_g:
            for gm in range(16):
                gw_t = gw_p.tile([128, 16, 128], BF, name="gw_t")
                nc.sync.dma_start(
                    gw_t[:],
                    gwT[:, gm * 128 : (gm + 1) * 128].rearrange(
                        "(ko p) g -> p ko g", p=128
                    ),
                )
                for gn in range(2):
                    pg = ps_g.tile([128, 512], F32, name="pg")
                    for gk in range(16):
                        nc.tensor.matmul(
                            pg[:],
                            gw_t[:, gk, :],
                            xT[:, gk, gn * 512 : (gn + 1) * 512],
                            start=(gk == 0),
                            stop=(gk == 15),
                        )
                    gg_t = gg_p.tile([128, 512], BF, name="gg_t")
                    nc.scalar.activation(gg_t[:], pg[:], AF.Sigmoid)
                    nc.sync.dma_start(
                        gate_spill[gm, :, gn * 512 : (gn + 1) * 512], gg_t[:]
                    )
        _xt_ctx.close()

        # ---- phase A: attention per head pair ----------------------------
        with tc.tile_pool(name="eg_p", bufs=2) as eg_p, tc.tile_pool(
            name="dd_p", bufs=2
        ) as dd_p, tc.tile_pool(name="gc_p", bufs=2) as gc_p, tc.tile_pool(
            name="qq_p", bufs=2
        ) as qq_p, tc.tile_pool(name="aw_p", bufs=2) as aw_p, tc.tile_pool(
            name="ss_p", bufs=2
        ) as ss_p, tc.tile_pool(name="sq_p", bufs=2) as sq_p, tc.tile_pool(
            name="ps_aw", bufs=2, space="PSUM"
        ) as ps_aw, tc.tile_pool(
            name="ps_ys", bufs=2, space="PSUM"
        ) as ps_ys, tc.tile_pool(
            name="ps_sq", bufs=1, space="PSUM"
        ) as ps_sq:
            ssq0 = ps_sq.tile([1, 512], F32, name="ssq0")
            ssq1 = ps_sq.tile([1, 512], F32, name="ssq1")
            for c in range(NP):
                hA, hB = 2 * c, 2 * c + 1
                eg_t = eg_p.tile([128, NC, D], F32, name="eg_t")
                nc.sync.dma_start(
                    eg_t[:], egath[:, c, :, :].rearrange("cc p e -> p cc e")
                )
                dd_t = dd_p.tile([128, 2, 2, BS], F32, name="dd_t")
                nc.sync.dma_start(dd_t[:], ddm[c])
                gate_c = gc_p.tile([128, T], BF, name="gate_c")
                nc.sync.dma_start(gate_c[:], gate_spill[c])
                # chunk-start state S0 (f32, both heads)
                sg0 = ss_p.tile([128, D], F32, name="sg0")
                nc.vector.tensor_scalar_mul(
                    sg0[:], eg_t[:, 0, :], swm_sb[:, c * 8 : c * 8 + 1]
                )
                for cc in range(1, NC):
                    nc.vector.scalar_tensor_tensor(
                        sg0[:],
                        eg_t[:, cc, :],
                        swm_sb[:, c * 8 + cc : c * 8 + cc + 1],
                        sg0[:],
                        ALU.mult,
                        ALU.add,
                    )
                # decayed queries
                qdq = qq_p.tile([128, NBLK, BS], BF, name="qdq")
                for jb in range(NBLK):
                    nc.vector.tensor_mul(
                        qdq[:, jb, :],
                        qT_sb[:, c, jb * BS : (jb + 1) * BS],
                        qdm_sb[:, c, :],
                    )
                for jb in range(NBLK):
                    # per-block start state (bf16 cast for PE)
                    ss_bf = ss_p.tile([128, D], BF, name="ss_bf")
                    if jb == 0:
                        nc.scalar.copy(ss_bf[:], sg0[:])
                    else:
                        ss_f = ss_p.tile([128, D], F32, name="ss_f")
                        nc.vector.scalar_tensor_tensor(
                            ss_f[:],
                            sg0[:],
                            bdm_sb[:, c, jb : jb + 1],
                            prefix_sb[:, c, jb - 1, :],
                            ALU.mult,
                            ALU.add,
                        )
                        nc.scalar.copy(ss_bf[:], ss_f[:])
                    # intra-block attention weights (paired row groups)
                    pawA = ps_aw.tile([128, 2, BS], F32, name="pawA")
                    pawB = ps_aw.tile([128, 2, BS], F32, name="pawB")
                    t0 = jb * BS
                    nc.tensor.matmul(
                        pawA[:, 0, :],
                        kT_sb[0:64, c, t0 : t0 + 128],
                        qT_sb[0:64, c, t0 : t0 + BS],
                        start=True, stop=True,
                    )
                    nc.tensor.matmul(
                        pawB[:, 0, :],
                        kT_sb[64:128, c, t0 : t0 + 128],
                        qT_sb[64:128, c, t0 : t0 + BS],
                        start=True, stop=True,
                    )
                    nc.tensor.matmul(
                        pawA[:, 1, 128:256],
                        kT_sb[0:64, c, t0 + 128 : t0 + 256],
                        qT_sb[0:64, c, t0 + 128 : t0 + 256],
                        start=True, stop=True,
                    )
                    nc.tensor.matmul(
                        pawB[:, 1, 128:256],
                        kT_sb[64:128, c, t0 + 128 : t0 + 256],
                        qT_sb[64:128, c, t0 + 128 : t0 + 256],
                        start=True, stop=True,
                    )
                    awmA = aw_p.tile([128, 2, BS], BF, name="awmA")
                    awmB = aw_p.tile([128, 2, BS], BF, name="awmB")
                    nc.vector.tensor_mul(
                        awmA[:, 0, :], pawA[:, 0, :], dd_t[:, 0, 0, :]
                    )
                    nc.vector.tensor_mul(
                        awmA[:, 1, 128:256], pawA[:, 1, 128:256],
                        dd_t[:, 0, 1, 128:256],
                    )
                    nc.vector.tensor_mul(
                        awmB[:, 0, :], pawB[:, 0, :], dd_t[:, 1, 0, :]
                    )
                    nc.vector.tensor_mul(
                        awmB[:, 1, 128:256], pawB[:, 1, 128:256],
                        dd_t[:, 1, 1, 128:256],
                    )
                    # ys = intra + inter (paired col groups)
                    pys = ps_ys.tile([128, BS], F32, name="pys",
                                     padded_shape=[128, 512])
                    nc.tensor.matmul(
                        pys[0:64, :], ss_bf[0:64, :], qdq[0:64, jb, :],
                        start=True, stop=False,
                    )
                    nc.tensor.matmul(
                        pys[64:128, :], ss_bf[64:128, :], qdq[64:128, jb, :],
                        start=True, stop=False,
                    )
                    nc.tensor.matmul(
                        pys[0:64, :],
                        V_sb[:, 2 * jb, hA * D : (hA + 1) * D],
                        awmA[:, 0, :],
                        start=False, stop=False,
                    )
                    nc.tensor.matmul(
                        pys[64:128, :],
                        V_sb[:, 2 * jb, hB * D : (hB + 1) * D],
                        awmB[:, 0, :],
                        start=False, stop=False,
                    )
                    nc.tensor.matmul(
                        pys[0:64, 128:256],
                        V_sb[:, 2 * jb + 1, hA * D : (hA + 1) * D],
                        awmA[:, 1, 128:256],
                        start=False, stop=True,
                    )
                    nc.tensor.matmul(
                        pys[64:128, 128:256],
                        V_sb[:, 2 * jb + 1, hB * D : (hB + 1) * D],
                        awmB[:, 1, 128:256],
                        start=False, stop=True,
                    )
                    nc.scalar.copy(ys_sb[:, c, t0 : t0 + BS], pys[:])
                # rmsnorm sum-of-squares + gate application (interleaved)
                sq_t = sq_p.tile([128, T], BF, name="sq_t")
                nc.scalar.activation(sq_t[:], ys_sb[:, c, :], AF.Square)
                nc.tensor.matmul(
                    ssq0[:], ones_b[:], sq_t[:, 0:512],
                    start=(c == 0), stop=(c == NP - 1), skip_group_check=True,
                )
                nc.tensor.matmul(
                    ssq1[:], ones_b[:], sq_t[:, 512:1024],
                    start=(c == 0), stop=(c == NP - 1), skip_group_check=True,
                )
                nc.vector.scalar_tensor_tensor(
                    ys_sb[:, c, :],
                    ys_sb[:, c, :],
                    nw_sb[:, c : c + 1],
                    gate_c[:],
                    ALU.mult,
                    ALU.mult,
                )
            # ssq round-trip for per-token layout
            with tc.tile_pool(name="ns_p", bufs=1) as ns_p:
                ssq_sb = ns_p.tile([1, T], F32, name="ssq_sb")
                nc.vector.tensor_copy(ssq_sb[:, 0:512], ssq0[:])
                nc.vector.tensor_copy(ssq_sb[:, 512:1024], ssq1[:])
                nc.sync.dma_start(ssq_rt[:], ssq_sb[:])
        _qd_ctx.close()
        _qk_ctx.close()

        # ---- phase F: output projection ----------------------------------
        with tc.tile_pool(name="nsv_p", bufs=1) as nsv_p:
            ns_l = nsv_p.tile([128, NCH], F32, name="ns_l")
            nc.sync.dma_start(ns_l[:], ssq_rt.rearrange("(c p) -> p c", p=128))
            ns_t = nsv_p.tile([128, NCH], F32, name="ns_t")
            nc.scalar.activation(
                ns_t[:], ns_l[:], AF.Sqrt, bias=eps_sb[:, 0:1], scale=1.0 / (H * D)
            )
            ns_sb = nsv_p.tile([128, NCH], F32, name="ns_sb")
            nc.vector.reciprocal(ns_sb[:], ns_t[:])

            with tc.tile_pool(name="ow_p", bufs=2) as ow_p, tc.tile_pool(
                name="oo_p", bufs=3
            ) as oo_p, tc.tile_pool(name="ps_o", bufs=3, space="PSUM") as ps_o:
                for n in range(4):
                    ow_t = ow_p.tile([128, 16, 512], BF, name="ow_t")
                    nc.sync.dma_start(
                        ow_t[:],
                        owT[:, n * 512 : (n + 1) * 512].rearrange(
                            "(ko p) f -> p ko f", p=128
                        ),
                    )
                    for m in range(NCH):
                        po = ps_o.tile([128, 512], F32, name="po")
                        for k in range(16):
                            nc.tensor.matmul(
                                po[:],
                                ys_sb[:, k, m * 128 : (m + 1) * 128],
                                ow_t[:, k, :],
                                start=(k == 0),
                                stop=(k == 15),
                            )
                        oo_t = oo_p.tile([128, 512], F32, name="oo_t")
                        nc.scalar.mul(oo_t[:], po[:], ns_sb[:, m : m + 1])
                        nc.sync.dma_start(
                            out[m * 128 : (m + 1) * 128, n * 512 : (n + 1) * 512],
                            oo_t[:],
                        )
        _ys_ctx.close()
        _pf_ctx.close()
        _v_ctx.close()
        free_bdm()
        free_swm()
        free_kdm()
        free_nw()
        free_eps()
        free_ones_b()
        free_ones_f()
        free_ident()
    nc.finalize()
    return nc


_CACHE = {}


def _get_nc():
    if "nc" not in _CACHE:
        _CACHE["nc"] = _build_nc()
    return _CACHE["nc"]


def _host_prep(hidden_states, qkv_w, out_w, gate_w, norm_w):
    slope, qd, kd, dd, bd = _decay()
    w3 = qkv_w.reshape(H, 3 * D, HID)
    wq = w3[:, 0:D, :].reshape(H * D, HID)
    wk = w3[:, D : 2 * D, :].reshape(H * D, HID)
    wv = w3[:, 2 * D : 3 * D, :].reshape(H * D, HID)
    wqT = np.ascontiguousarray(wq.T.astype(BF_NP))
    wkT = np.ascontiguousarray(wk.T.astype(BF_NP))
    wvT = np.ascontiguousarray(wv.T.astype(BF_NP))
    gwT = np.ascontiguousarray(gate_w.T.astype(BF_NP))
    owT = np.ascontiguousarray(out_w.T.astype(BF_NP))
    # ddm[c, p, e, jc, i] = dd[2c+e, i, jc*128+p]
    ddm = np.ascontiguousarray(
        dd.reshape(NP, 2, BS, 2, 128).transpose(0, 4, 1, 3, 2).astype(np.float32)
    )
    # qdm[p, c, i] = qd[2c + (p>=64), i]
    qdm_pair = qd.reshape(NP, 2, BS)                       # (c, e, i)
    qdm = np.ascontiguousarray(
        np.broadcast_to(
            qdm_pair.transpose(1, 0, 2)[:, None, :, :], (2, 64, NP, BS)
        ).reshape(128, NP, BS).astype(BF_NP)
    )
    kdm = np.ascontiguousarray(
        kd.reshape(H, 2, 128).transpose(2, 0, 1).reshape(128, 2 * H).astype(np.float32)
    )
    nw = np.ascontiguousarray(norm_w.reshape(16, 128).T.astype(np.float32))
    # bdm[p, c, jb] = bd[2c + (p>=64)]^jb
    jbp = np.arange(NBLK, dtype=np.float64)
    bdp = bd[:, None] ** jbp[None, :]                      # (H, NBLK)
    bdm = np.ascontiguousarray(
        np.broadcast_to(
            bdp.reshape(NP, 2, NBLK).transpose(1, 0, 2)[:, None, :, :],
            (2, 64, NP, NBLK),
        ).reshape(128, NP, NBLK).astype(np.float32)
    )

    shared = dict(wqT=wqT, wkT=wkT, wvT=wvT, gwT=gwT, owT=owT, ddm=ddm,
                  qdm=qdm, kdm=kdm, nw=nw, bdm=bdm)
    in_maps = []
    for c in range(NC):
        bb, p = c // 4, c % 4
        hsT = np.ascontiguousarray(
            hidden_states[bb, p * T : (p + 1) * T, :].T.astype(BF_NP)
        )
        # swm[p_, c_*8+cc] = weight of core cc for head 2c_+(p_>=64)
        sw = np.zeros((H, NC), dtype=np.float64)
        for cc in range(NC):
            if cc // 4 == bb and cc % 4 < p:
                sw[:, cc] = bd ** (4.0 * (p - 1 - (cc % 4)))
        swm = np.ascontiguousarray(
            np.broadcast_to(
                sw.reshape(NP, 2, NC).transpose(1, 0, 2)[:, None, :, :],
                (2, 64, NP, NC),
            ).reshape(128, NP * 8).astype(np.float32)
        )
        in_maps.append(dict(hsT=hsT, swm=swm, **shared))
    return in_maps


def _run(inputs, trace=False):
    nc = _get_nc()
    in_maps = _host_prep(
        np.asarray(inputs["hidden_states"], dtype=np.float32),
        np.asarray(inputs["qkv_w"], dtype=np.float32),
        np.asarray(inputs["out_w"], dtype=np.float32),
        np.asarray(inputs["gate_w"], dtype=np.float32),
        np.asarray(inputs["norm_w"], dtype=np.float32),
    )
    res = run_bass_kernel_spmd(nc, in_maps, core_ids=list(range(NC)), trace=trace)
    full = np.empty((B, S, HID), dtype=np.float32)
    for c in range(NC):
        bb, p = c // 4, c % 4
        full[bb, p * T : (p + 1) * T, :] = res.results[c]["out"]
    return full, res


def kernel(**inputs):
    return _run(inputs, trace=False)[0]


def kernel_traced(**inputs):
    full, res = _run(inputs, trace=True)
    return full, res.exec_time_ns
